# revision 6
# baseline (speedup 1.0000x reference)
"""KMeans summarize kernel for Trainium2 — v3: Relu one-hot, balanced engines.

Per 128-point tile: bf16 dist matmul -> PSUM q = CC - 2Xc.
Mins: DVE tensor_reduce over 3-tile batches (amortizes PSUM init).
Bias prep: per 6 tiles, one DVE tensor_scalar computes b = m/delta + 1.
One-hot (all {1,0} fp8, single PSUM scatter accumulator):
  A-tiles (most): ACT Relu((m - q + delta)/delta) -> exact 1 at argmin,
     0 elsewhere, fractional split for ties within delta.
  V-tiles (balance knob): DVE is_le(q, m) straight from PSUM (exact).
Scatter: fp8 DoubleRow matmul per pair of tiles into one accumulator.
Inertia from per-tile mins + XX shipped separately.
"""

import sys

sys.path.insert(0, "/opt/trn_rl_repo")

import numpy as np

N, D, K = 500_000, 64, 512
NCORES = 8
P = 128
TILES = 492                      # per-core tiles; divisible by 4
PAIRS = TILES // 2               # 246
ROWS = P * TILES                 # 62976 per core
NPAD = NCORES * ROWS             # 503808
GROUP = 12                       # tiles per xt DMA slab (41 slabs)
MGROUP = 6                       # pairs per mh DMA slab (41 slabs)
CROWS = D + 2                    # dist contraction: X(64) + 1 + 1

INVD = 1024.0                    # 1/delta
V_EVERY = 16                     # tile t is V-type iff t % V_EVERY == V_PHASE
V_PHASE = 8
PEND_HI = 8                      # flush 4 scatter pairs when 8 pending
FLUSH_N = 4

_CACHE = {}


def _is_v(t: int) -> bool:
    return t % V_EVERY == V_PHASE


def _build():
    import concourse.bass as bass
    import concourse.mybir as mybir
    import concourse.tile as tile

    fp32 = mybir.dt.float32
    bf16 = mybir.dt.bfloat16
    fp8 = mybir.dt.float8e4

    nc = bass.Bass()
    xt = nc.dram_tensor("xt", (CROWS, ROWS), bf16, kind="ExternalInput")
    rhs = nc.dram_tensor("rhs", (CROWS, K), bf16, kind="ExternalInput")
    mh8 = nc.dram_tensor("mh8", (P, PAIRS, 2, 2 * D), fp8, kind="ExternalInput")
    xxb = nc.dram_tensor("xxb", (P, TILES), fp32, kind="ExternalInput")
    out = nc.dram_tensor("out", (P, K + 4), fp32, kind="ExternalOutput")

    AX = mybir.AxisListType.X
    OP = mybir.AluOpType
    AF = mybir.ActivationFunctionType
    DR = mybir.MatmulPerfMode.DoubleRow

    from contextlib import ExitStack

    with tile.TileContext(nc) as tc, ExitStack() as es:
        consts = es.enter_context(tc.tile_pool(name="consts", bufs=1))
        xtpA = es.enter_context(tc.tile_pool(name="xtpA", bufs=2))
        xtpB = es.enter_context(tc.tile_pool(name="xtpB", bufs=2))
        mhpA = es.enter_context(tc.tile_pool(name="mhpA", bufs=2))
        mhpB = es.enter_context(tc.tile_pool(name="mhpB", bufs=2))
        a8p = es.enter_context(tc.tile_pool(name="a8p", bufs=10))
        dpp = es.enter_context(tc.tile_pool(name="dpp", bufs=3, space="PSUM"))
        scp = es.enter_context(tc.tile_pool(name="scp", bufs=1, space="PSUM"))

        rhs_sb = consts.tile([CROWS, K], bf16)
        nc.sync.dma_start(out=rhs_sb, in_=rhs.ap())
        xxb_sb = consts.tile([P, TILES], fp32)
        nc.sync.dma_start(out=xxb_sb, in_=xxb.ap())
        mbuf = consts.tile([P, TILES], fp32)
        bbuf = consts.tile([P, TILES], fp32)

        scat = scp.tile([P, K], fp32)          # single {1,0} accumulator

        n_xslab = (TILES + GROUP - 1) // GROUP
        n_mslab = (PAIRS + MGROUP - 1) // MGROUP
        xt_slabs = [None] * n_xslab
        mh_slabs = [None] * n_mslab

        pend = []        # (mh_sb, pj, a8pair, pr)

        def flush_pend(k):
            for _ in range(k):
                pmh, ppj, pa8, ppr = pend.pop(0)
                nc.tensor.matmul(
                    scat, pmh[:, ppj, :, :], pa8,
                    start=(ppr == 0), stop=(ppr == PAIRS - 1),
                    perf_mode=DR,
                )

        def get_xslab(t):
            gx = t // GROUP
            if xt_slabs[gx] is None:
                g = min(GROUP, TILES - gx * GROUP)
                xtg = (xtpA if gx % 2 == 0 else xtpB).tile(
                    [CROWS, GROUP * P], bf16, tag="xtg", name="xtg"
                )
                src = bass.AP(
                    tensor=xt, offset=gx * GROUP * P,
                    ap=[[ROWS, CROWS], [1, g * P]],
                )
                nc.sync.dma_start(out=xtg[:, : g * P], in_=src)
                xt_slabs[gx] = xtg
            return xt_slabs[gx], t - gx * GROUP

        def get_mslab(pr):
            gm = pr // MGROUP
            if mh_slabs[gm] is None:
                g = min(MGROUP, PAIRS - gm * MGROUP)
                mhg = (mhpA if gm % 2 == 0 else mhpB).tile(
                    [P, MGROUP, 2, 2 * D], fp8, tag="mhg", name="mhg"
                )
                src = bass.AP(
                    tensor=mh8, offset=gm * MGROUP * 2 * 2 * D,
                    ap=[[PAIRS * 2 * 2 * D, P], [1, g * 2 * 2 * D]],
                )
                nc.sync.dma_start(out=mhg[:, :g, :, :], in_=src)
                mh_slabs[gm] = mhg
            return mh_slabs[gm], pr - gm * MGROUP

        dd_open = {}     # pair -> dist PSUM tile

        for pr in range(PAIRS):
            t0 = 2 * pr
            dd = dpp.tile([P, 2, K], fp32, tag="dist")
            dd_open[pr] = dd
            for h in (0, 1):
                t = t0 + h
                xtg, xj = get_xslab(t)
                nc.tensor.matmul(
                    dd[:, h, :], xtg[:, xj * P:(xj + 1) * P], rhs_sb,
                    start=True, stop=True,
                )
            nc.vector.tensor_reduce(
                out=mbuf[:, t0:t0 + 2], in_=dd, axis=AX, op=OP.min,
            )
            if pr % 2 == 0:
                continue
            # bias prep for the 4 tiles of pairs pr-1, pr; then emit their
            # one-hots and scatter matmuls (3-deep dist pipeline).
            b0 = 2 * (pr - 1)
            nc.vector.tensor_scalar(
                out=bbuf[:, b0:b0 + 4], in0=mbuf[:, b0:b0 + 4],
                scalar1=INVD, scalar2=1.0,
                op0=OP.mult, op1=OP.add,
            )
            for ep in (pr - 1, pr):
                edd = dd_open.pop(ep)
                a8 = a8p.tile([P, 2, K], fp8, tag="a8", name="a8")
                for h in (0, 1):
                    t = 2 * ep + h
                    if _is_v(t):
                        nc.vector.tensor_scalar(
                            out=a8[:, h, :], in0=edd[:, h, :],
                            scalar1=mbuf[:, t:t + 1], scalar2=None,
                            op0=OP.is_le,
                        )
                    else:
                        nc.scalar.activation(
                            out=a8[:, h, :], in_=edd[:, h, :], func=AF.Relu,
                            bias=bbuf[:, t:t + 1], scale=-INVD,
                        )
                mhg, pj = get_mslab(ep)
                pend.append((mhg, pj, a8, ep))
                if len(pend) >= PEND_HI:
                    flush_pend(FLUSH_N)
        flush_pend(len(pend))

        # ---- finalize ----
        out_sb = consts.tile([P, K + 4], fp32)
        nc.vector.tensor_copy(out_sb[:, :K], scat)
        vbuf = consts.tile([P, TILES], fp32)
        nc.vector.tensor_tensor(
            out=vbuf, in0=mbuf, in1=xxb_sb, op=OP.add
        )
        nc.vector.tensor_scalar_max(vbuf, vbuf, 0.0)
        vsq = consts.tile([P, TILES], fp32)
        nc.scalar.activation(
            out=vsq, in_=vbuf, func=AF.Sqrt, scale=1.0 / D,
            accum_out=out_sb[:, K:K + 1],
        )
        nc.vector.memset(out_sb[:, K + 1:], 0.0)
        nc.sync.dma_start(out=out.ap(), in_=out_sb)

    _split_multi_waits(nc, mybir)
    return nc


def _split_multi_waits(nc, mybir):
    """Walrus allows max 1 sem-wait per instruction: hoist extras onto
    inserted NoOps on the same engine queue."""
    import copy

    module = nc.m
    new_module = copy.replace(module, functions=[])
    for function in module.functions:
        new_function = copy.replace(function, blocks=[])
        new_function.set_allocations_from_list(function.allocations)
        for block in function.blocks:
            new_insts = []
            for ins in block.instructions:
                si = ins.sync_info
                if si is not None and si.on_wait and len(si.on_wait) > 1:
                    waits = list(si.on_wait)
                    for k, w in enumerate(waits[:-1]):
                        new_insts.append(mybir.InstNoOp(
                            name=f"{ins.name}-wsplit{k}", engine=ins.engine,
                            ins=[], outs=[],
                            sync_info=mybir.SyncInfo(on_wait=[w], on_update=[]),
                        ))
                    ins.sync_info = mybir.SyncInfo(
                        on_wait=[waits[-1]], on_update=list(si.on_update or [])
                    )
                new_insts.append(ins)
            new_function.blocks.append(copy.replace(block, instructions=new_insts))
        new_module.functions.append(new_function)
    nc.m = new_module


def _prep_inputs(X, centroids, sample_weight):
    import ml_dtypes

    bf16 = ml_dtypes.bfloat16
    f8 = ml_dtypes.float8_e4m3

    C = np.asarray(centroids, dtype=np.float32)
    X = np.asarray(X, dtype=np.float32)
    W = np.asarray(sample_weight, dtype=np.float32)

    CC = (C * C).sum(axis=1)
    CChi = CC.astype(bf16)
    CClo = (CC - CChi.astype(np.float32)).astype(bf16)
    rhs = np.empty((CROWS, K), dtype=bf16)
    rhs[:D] = (-2.0 * C.T).astype(bf16)
    rhs[D] = CChi
    rhs[D + 1] = CClo

    Xp = np.empty((NPAD, D), dtype=np.float32)
    Xp[:N] = X
    Xp[N:] = C[0]
    Wp = np.zeros((NPAD, D), dtype=np.float32)
    Wp[:N] = W
    XXp = np.einsum("ij,ij->i", Xp, Xp)

    Mh = np.empty((NPAD, 2 * D), dtype=f8)
    Mh[:, :D] = (Xp * Wp).astype(f8)
    Mh[:, D:] = Wp.astype(f8)

    Xb = Xp.astype(bf16)
    in_maps = []
    for c in range(NCORES):
        sl = slice(c * ROWS, (c + 1) * ROWS)
        xtc = np.empty((CROWS, ROWS), dtype=bf16)
        xtc[:D] = Xb[sl].T
        xtc[D] = bf16(1.0)
        xtc[D + 1] = bf16(1.0)
        mh_c = np.ascontiguousarray(
            Mh[sl].reshape(PAIRS, 2, P, 2 * D).transpose(2, 0, 1, 3)
        )
        xx_c = np.ascontiguousarray(
            XXp[sl].reshape(TILES, P).T.astype(np.float32)
        )
        in_maps.append({"xt": xtc, "rhs": rhs, "mh8": mh_c, "xxb": xx_c})
    return in_maps


def run(X, centroids, sample_weight, trace=False):
    from concourse.bass_utils import run_bass_kernel_spmd

    if "nc" not in _CACHE:
        _CACHE["nc"] = _build()
    in_maps = _prep_inputs(X, centroids, sample_weight)
    res = run_bass_kernel_spmd(
        _CACHE["nc"], in_maps, core_ids=list(range(NCORES)), trace=trace
    )
    xw = np.zeros((K, D), dtype=np.float64)
    ws = np.zeros((K, D), dtype=np.float64)
    inertia = 0.0
    for c in range(NCORES):
        o = res.results[c]["out"]
        xw += o[:D, :K].T.astype(np.float64)
        ws += o[D:2 * D, :K].T.astype(np.float64)
        inertia += float(o[:, K].sum(dtype=np.float64))
    packed = np.concatenate(
        [xw, ws, np.full((1, D), inertia)], axis=0
    ).astype(np.float32)
    return packed, res


def kernel(X, centroids, sample_weight):
    packed, _ = run(X, centroids, sample_weight)
    return packed


# revision 9
# speedup vs baseline: 1.7404x; 1.7404x over previous
"""KMeans summarize kernel for Trainium2 — v5: Sigmoid one-hot, fp8-DR dist.

Distances: fp8 DoubleRow matmul computes q' = 1024*(CC - 2Xc) via
error-compensated fp8 splits (X*64 hi/lo, -2C*16 hi/lo, CC via 6 fp8
sub-rows), 99 contraction partitions, 2 k-tiles (DoubleRow).
Per pair of 128-point tiles: DVE tensor_reduce min -> m'.
One-hot ({0.5, 0} fp8, no bias prep needed):
  A-tiles: ACT Sigmoid(m' - q') -> exactly 0.5 at argmin, 0 elsewhere
     (the 1024 sharpening makes the transition width ~1% of typical gaps;
      symmetric ties split conservatively: sum stays 0.5).
  V-tiles (balance knob): DVE (q' <= m') * 0.5 straight from PSUM.
Scatter: fp8 DoubleRow matmul per pair into one PSUM accumulator;
finalize doubles it. Inertia from m'/1024 + XX.
"""

import sys

sys.path.insert(0, "/opt/trn_rl_repo")

import numpy as np

N, D, K = 500_000, 64, 512
NCORES = 8
P = 128
TILES = 492                      # per-core tiles (even)
PAIRS = TILES // 2               # 246
ROWS = P * TILES                 # 62976 per core
NPAD = NCORES * ROWS             # 503808
GROUP = 12                       # tiles per xt DMA slab (41 slabs)
MGROUP = 6                       # pairs per mh DMA slab (41 slabs)
CROWS = 99                       # DR contraction partitions (2 sub-rows each)

XSCALE = 64.0                    # X side scale (clip at +-240 in fp8)
RSCALE = 16.0                    # -2C side scale
QSCALE = XSCALE * RSCALE         # q' = 1024 * q
CCW = 192.0                      # lhsT weight for the CC sub-rows

V_EVERY = 16                     # tile t is V-type iff t % V_EVERY == V_PHASE
V_PHASE = 8
PEND_HI = 8
FLUSH_N = 4

_CACHE = {}


def _is_v(t: int) -> bool:
    return t % V_EVERY == V_PHASE


def _build():
    import concourse.bass as bass
    import concourse.mybir as mybir
    import concourse.tile as tile

    fp32 = mybir.dt.float32
    fp8 = mybir.dt.float8e4

    nc = bass.Bass()
    xt = nc.dram_tensor("xt", (CROWS, 2, ROWS), fp8, kind="ExternalInput")
    rhs = nc.dram_tensor("rhs", (CROWS, 2, K), fp8, kind="ExternalInput")
    mh8 = nc.dram_tensor("mh8", (P, PAIRS, 2, 2 * D), fp8, kind="ExternalInput")
    xxb = nc.dram_tensor("xxb", (P, TILES), fp32, kind="ExternalInput")
    out = nc.dram_tensor("out", (P, K + 4), fp32, kind="ExternalOutput")

    AX = mybir.AxisListType.X
    OP = mybir.AluOpType
    AF = mybir.ActivationFunctionType
    DR = mybir.MatmulPerfMode.DoubleRow

    from contextlib import ExitStack

    with tile.TileContext(nc) as tc, ExitStack() as es:
        consts = es.enter_context(tc.tile_pool(name="consts", bufs=1))
        xtpA = es.enter_context(tc.tile_pool(name="xtpA", bufs=2))
        xtpB = es.enter_context(tc.tile_pool(name="xtpB", bufs=2))
        mhpA = es.enter_context(tc.tile_pool(name="mhpA", bufs=2))
        mhpB = es.enter_context(tc.tile_pool(name="mhpB", bufs=2))
        a8p = es.enter_context(tc.tile_pool(name="a8p", bufs=10))
        dpp = es.enter_context(tc.tile_pool(name="dpp", bufs=3, space="PSUM"))
        scp = es.enter_context(tc.tile_pool(name="scp", bufs=1, space="PSUM"))

        rhs_sb = consts.tile([CROWS, 2, K], fp8)
        nc.sync.dma_start(out=rhs_sb, in_=rhs.ap())
        xxb_sb = consts.tile([P, TILES], fp32)
        nc.sync.dma_start(out=xxb_sb, in_=xxb.ap())
        mbuf = consts.tile([P, TILES], fp32)

        scat = scp.tile([P, K], fp32)          # {0.5, 0} accumulator

        n_xslab = (TILES + GROUP - 1) // GROUP
        n_mslab = (PAIRS + MGROUP - 1) // MGROUP
        xt_slabs = [None] * n_xslab
        mh_slabs = [None] * n_mslab

        pend = []        # (mh_sb, pj, a8pair, pr)

        def flush_pend(k):
            for _ in range(k):
                pmh, ppj, pa8, ppr = pend.pop(0)
                nc.tensor.matmul(
                    scat, pmh[:, ppj, :, :], pa8,
                    start=(ppr == 0), stop=(ppr == PAIRS - 1),
                    perf_mode=DR,
                )

        def get_xslab(t):
            gx = t // GROUP
            if xt_slabs[gx] is None:
                g = min(GROUP, TILES - gx * GROUP)
                xtg = (xtpA if gx % 2 == 0 else xtpB).tile(
                    [CROWS, 2, GROUP * P], fp8, tag="xtg", name="xtg"
                )
                src = bass.AP(
                    tensor=xt, offset=gx * GROUP * P,
                    ap=[[2 * ROWS, CROWS], [ROWS, 2], [1, g * P]],
                )
                nc.sync.dma_start(out=xtg[:, :, : g * P], in_=src)
                xt_slabs[gx] = xtg
            return xt_slabs[gx], t - gx * GROUP

        def get_mslab(pr):
            gm = pr // MGROUP
            if mh_slabs[gm] is None:
                g = min(MGROUP, PAIRS - gm * MGROUP)
                mhg = (mhpA if gm % 2 == 0 else mhpB).tile(
                    [P, MGROUP, 2, 2 * D], fp8, tag="mhg", name="mhg"
                )
                src = bass.AP(
                    tensor=mh8, offset=gm * MGROUP * 2 * 2 * D,
                    ap=[[PAIRS * 2 * 2 * D, P], [1, g * 2 * 2 * D]],
                )
                nc.sync.dma_start(out=mhg[:, :g, :, :], in_=src)
                mh_slabs[gm] = mhg
            return mh_slabs[gm], pr - gm * MGROUP

        for pr in range(PAIRS):
            t0 = 2 * pr
            dd = dpp.tile([P, 2, K], fp32, tag="dist")
            for h in (0, 1):
                t = t0 + h
                xtg, xj = get_xslab(t)
                nc.tensor.matmul(
                    dd[:, h, :], xtg[:, :, xj * P:(xj + 1) * P], rhs_sb,
                    start=True, stop=True, perf_mode=DR,
                )
            nc.vector.tensor_reduce(
                out=mbuf[:, t0:t0 + 2], in_=dd, axis=AX, op=OP.min,
            )
            a8 = a8p.tile([P, 2, K], fp8, tag="a8", name="a8")
            for h in (0, 1):
                t = t0 + h
                if _is_v(t):
                    nc.vector.tensor_scalar(
                        out=a8[:, h, :], in0=dd[:, h, :],
                        scalar1=mbuf[:, t:t + 1], scalar2=0.5,
                        op0=OP.is_le, op1=OP.mult,
                    )
                else:
                    nc.scalar.activation(
                        out=a8[:, h, :], in_=dd[:, h, :], func=AF.Sigmoid,
                        bias=mbuf[:, t:t + 1], scale=-1.0,
                    )
            mhg, pj = get_mslab(pr)
            pend.append((mhg, pj, a8, pr))
            if len(pend) >= PEND_HI:
                flush_pend(FLUSH_N)
        flush_pend(len(pend))

        # ---- finalize ----
        out_sb = consts.tile([P, K + 4], fp32)
        nc.vector.tensor_scalar_mul(out_sb[:, :K], scat, 2.0)
        vbuf = consts.tile([P, TILES], fp32)
        nc.vector.tensor_tensor(
            out=vbuf, in0=mbuf, in1=xxb_sb, op=OP.add
        )
        nc.vector.tensor_scalar_max(vbuf, vbuf, 0.0)
        vsq = consts.tile([P, TILES], fp32)
        nc.scalar.activation(
            out=vsq, in_=vbuf, func=AF.Sqrt, scale=1.0 / (D * QSCALE),
            accum_out=out_sb[:, K:K + 1],
        )
        nc.vector.memset(out_sb[:, K + 1:], 0.0)
        nc.sync.dma_start(out=out.ap(), in_=out_sb)

    _split_multi_waits(nc, mybir)
    return nc


def _split_multi_waits(nc, mybir):
    """Walrus allows max 1 sem-wait per instruction: hoist extras onto
    inserted NoOps on the same engine queue."""
    import copy

    module = nc.m
    new_module = copy.replace(module, functions=[])
    for function in module.functions:
        new_function = copy.replace(function, blocks=[])
        new_function.set_allocations_from_list(function.allocations)
        for block in function.blocks:
            new_insts = []
            for ins in block.instructions:
                si = ins.sync_info
                if si is not None and si.on_wait and len(si.on_wait) > 1:
                    waits = list(si.on_wait)
                    for k, w in enumerate(waits[:-1]):
                        new_insts.append(mybir.InstNoOp(
                            name=f"{ins.name}-wsplit{k}", engine=ins.engine,
                            ins=[], outs=[],
                            sync_info=mybir.SyncInfo(on_wait=[w], on_update=[]),
                        ))
                    ins.sync_info = mybir.SyncInfo(
                        on_wait=[waits[-1]], on_update=list(si.on_update or [])
                    )
                new_insts.append(ins)
            new_function.blocks.append(copy.replace(block, instructions=new_insts))
        new_module.functions.append(new_function)
    nc.m = new_module


def _f8(a):
    import ml_dtypes
    return a.astype(ml_dtypes.float8_e4m3)


def _prep_inputs(X, centroids, sample_weight):
    import ml_dtypes

    f8 = ml_dtypes.float8_e4m3

    C = np.asarray(centroids, dtype=np.float32)
    X = np.asarray(X, dtype=np.float32)
    W = np.asarray(sample_weight, dtype=np.float32)

    # rhs fp8 DR layout: (99, 2, K)
    R = (-2.0 * RSCALE) * C.T                      # (D, K), |R| <~ 150
    Rhi = _f8(R)
    Rlo = _f8(R - Rhi.astype(np.float32))
    CCt = QSCALE * (C * C).sum(axis=1) / CCW       # target sum of 6 sub-rows
    rhs = np.zeros((CROWS, 2, K), dtype=f8)
    rhs[:D, 0] = Rhi
    rhs[:D, 1] = Rlo
    # partitions 64..95 carry Rhi for the interleaved Xlo sub-rows
    rhs[D:D + 32, 0] = Rhi[0::2]
    rhs[D:D + 32, 1] = Rhi[1::2]
    # CC sub-rows at partitions 96..98 (weights CCW on the lhsT side)
    r = CCt.astype(np.float64)
    for i in range(6):
        v = _f8(np.clip(r, -240, 240).astype(np.float32))
        rhs[D + 32 + i // 2, i % 2] = v
        r = r - v.astype(np.float64)

    Xp = np.empty((NPAD, D), dtype=np.float32)
    Xp[:N] = X
    Xp[N:] = C[0]
    Wp = np.zeros((NPAD, D), dtype=np.float32)
    Wp[:N] = W
    XXp = QSCALE * np.einsum("ij,ij->i", Xp, Xp)

    Xs = np.clip(XSCALE * Xp, -240.0, 240.0)
    Xhi = _f8(Xs)
    Xlo = _f8(Xs - Xhi.astype(np.float32))

    Mh = np.empty((NPAD, 2 * D), dtype=f8)
    Mh[:, :D] = _f8(Xp * Wp)
    Mh[:, D:] = _f8(Wp)

    in_maps = []
    for c in range(NCORES):
        sl = slice(c * ROWS, (c + 1) * ROWS)
        xtc = np.zeros((CROWS, 2, ROWS), dtype=f8)
        xtc[:D, 0] = Xhi[sl].T
        xtc[:D, 1] = Xhi[sl].T
        xlo = Xlo[sl].T                            # (D, ROWS)
        xtc[D:D + 32, 0] = xlo[0::2]
        xtc[D:D + 32, 1] = xlo[1::2]
        # Rhi rows for the Xlo partitions live in rhs: fix them up there once
        xtc[D + 32:D + 35, :, :] = f8(CCW)
        mh_c = np.ascontiguousarray(
            Mh[sl].reshape(PAIRS, 2, P, 2 * D).transpose(2, 0, 1, 3)
        )
        xx_c = np.ascontiguousarray(
            XXp[sl].reshape(TILES, P).T.astype(np.float32)
        )
        in_maps.append({"xt": xtc, "rhs": rhs, "mh8": mh_c, "xxb": xx_c})
    return in_maps


def run(X, centroids, sample_weight, trace=False):
    from concourse.bass_utils import run_bass_kernel_spmd

    if "nc" not in _CACHE:
        _CACHE["nc"] = _build()
    in_maps = _prep_inputs(X, centroids, sample_weight)
    res = run_bass_kernel_spmd(
        _CACHE["nc"], in_maps, core_ids=list(range(NCORES)), trace=trace
    )
    xw = np.zeros((K, D), dtype=np.float64)
    ws = np.zeros((K, D), dtype=np.float64)
    inertia = 0.0
    for c in range(NCORES):
        o = res.results[c]["out"]
        xw += o[:D, :K].T.astype(np.float64)
        ws += o[D:2 * D, :K].T.astype(np.float64)
        inertia += float(o[:, K].sum(dtype=np.float64))
    packed = np.concatenate(
        [xw, ws, np.full((1, D), inertia)], axis=0
    ).astype(np.float32)
    return packed, res


def kernel(X, centroids, sample_weight):
    packed, _ = run(X, centroids, sample_weight)
    return packed
